# revision 7
# baseline (speedup 1.0000x reference)
"""Trainium2 Bass kernel for GQA attention block (B=2,S=1024,D=4096,H=32,KVH=8,HD=128).

Tensor-parallel over heads across 8 NeuronCores: core c owns q-heads
[4c,4c+4), kv-head c, wo rows [512c, 512(c+1)). Attention is fully local
per core; host sums the 8 partial output projections and concatenates the
attention-probability shards over the head axis.
"""

import sys

sys.path.insert(0, "/opt/trn_rl_repo")

import math

import numpy as np
import ml_dtypes

import concourse.bass as bass
import concourse.bacc as bacc
import concourse.tile as tile
from concourse import mybir
from concourse.bass_utils import run_bass_kernel_spmd

BF16 = np.dtype(ml_dtypes.bfloat16)
F32 = mybir.dt.float32
BF = mybir.dt.bfloat16

B, S, D = 2, 1024, 4096
H, KVH, HD = 32, 8, 128
NC = 8
HPC = H // NC          # q heads per core = 4
EPS = 1e-5
SCALE = 1.0 / math.sqrt(HD)

Exp = mybir.ActivationFunctionType.Exp
Sqrt = mybir.ActivationFunctionType.Sqrt
Copy = mybir.ActivationFunctionType.Copy
ADD = mybir.AluOpType.add
MULT = mybir.AluOpType.mult
MAX = mybir.AluOpType.max
AXX = mybir.AxisListType.X


def build_nc():
    nc = bacc.Bacc("TRN2", target_bir_lowering=False, debug=False,
                   enable_asserts=True, num_devices=NC)

    xt = nc.dram_tensor("xt", [D, B * S], BF, kind="ExternalInput").ap()
    wq = nc.dram_tensor("wq", [D, HPC * HD], BF, kind="ExternalInput").ap()
    wk = nc.dram_tensor("wk", [D, HD], BF, kind="ExternalInput").ap()
    wv = nc.dram_tensor("wv", [D, HD], BF, kind="ExternalInput").ap()
    wo = nc.dram_tensor("wo", [HPC * HD, D], BF, kind="ExternalInput").ap()
    cost = nc.dram_tensor("cost", [HD, S], F32, kind="ExternalInput").ap()
    sint = nc.dram_tensor("sint", [HD, S], F32, kind="ExternalInput").ap()
    maskd = nc.dram_tensor("maskd", [128, 128], F32, kind="ExternalInput").ap()
    pswap = nc.dram_tensor("pswap", [128, 128], BF, kind="ExternalInput").ap()
    ident = nc.dram_tensor("ident", [128, 128], BF, kind="ExternalInput").ap()
    onescol = nc.dram_tensor("onescol", [128, 1], BF, kind="ExternalInput").ap()
    onesrow = nc.dram_tensor("onesrow", [1, 128], BF, kind="ExternalInput").ap()
    chain = nc.dram_tensor("chain", [128, 4], F32, kind="ExternalInput").ap()

    attn_e = nc.dram_tensor("attn", [B, HPC, S, S], F32, kind="ExternalOutput").ap()
    outp_e = nc.dram_tensor("outp", [B * S, D], F32, kind="ExternalOutput").ap()
    chain_o = nc.dram_tensor("chain_out", [128, 4], F32, kind="ExternalOutput").ap()

    NB = S // 128          # 8 l/s blocks per batch
    NDC = D // 128         # 32 contraction chunks
    CH = 512               # seq chunk for projections
    NCH = B * S // CH      # 4 chunks

    with tile.TileContext(nc) as tc:
        with tc.tile_pool(name="chainp", bufs=1) as chp:
            cht = chp.tile([128, 4], F32, name="cht")
            nc.sync.dma_start(cht[:], chain[:])
            nc.sync.dma_start(chain_o[:], cht[:])

        with tc.tile_pool(name="persist", bufs=1) as pp:
            qn = {(b, h): pp.tile([128, S], BF, name=f"qn_{b}_{h}", tag=f"qn_{b}_{h}")
                  for b in range(B) for h in range(HPC)}
            kn = {b: pp.tile([128, S], BF, name=f"kn_{b}", tag=f"kn_{b}") for b in range(B)}
            vT = {b: pp.tile([128, S], BF, name=f"vT_{b}", tag=f"vT_{b}") for b in range(B)}
            vnat = {b: pp.tile([128, S], BF, name=f"vnat_{b}", tag=f"vnat_{b}") for b in range(B)}
            ctxT = {(b, h): pp.tile([128, S], BF, name=f"ctxT_{b}_{h}", tag=f"ctxT_{b}_{h}")
                    for b in range(B) for h in range(HPC)}

            # ---------------- Phase 1: projections + RoPE + QK rmsnorm -------
            with (
                tc.tile_pool(name="xtp", bufs=64) as xtp,
                tc.tile_pool(name="wp", bufs=1) as wp,
                tc.tile_pool(name="tbl", bufs=1) as tblp,
                tc.tile_pool(name="ep", bufs=3) as ep,
                tc.tile_pool(name="praw", bufs=2, space="PSUM") as praw_p,
                tc.tile_pool(name="paux", bufs=2, space="PSUM") as paux_p,
                tc.tile_pool(name="psml", bufs=2, space="PSUM") as psml_p,
            ):
                wq_sb = wp.tile([128, NDC * HPC * HD], BF, name="wq_sb")
                wk_sb = wp.tile([128, NDC * HD], BF, name="wk_sb")
                wv_sb = wp.tile([128, NDC * HD], BF, name="wv_sb")
                for d in range(NDC):
                    nc.sync.dma_start(wq_sb[:, d * 512:(d + 1) * 512],
                                      wq[d * 128:(d + 1) * 128, :])
                    nc.sync.dma_start(wk_sb[:, d * 128:(d + 1) * 128],
                                      wk[d * 128:(d + 1) * 128, :])
                    nc.sync.dma_start(wv_sb[:, d * 128:(d + 1) * 128],
                                      wv[d * 128:(d + 1) * 128, :])
                cosT = tblp.tile([128, S], F32, name="cosT")
                sinT = tblp.tile([128, S], F32, name="sinT")
                pswap_sb = tblp.tile([128, 128], BF, name="pswap_sb")
                onescol_sb = tblp.tile([128, 1], BF, name="onescol_sb")
                onesrow_sb = tblp.tile([1, 128], BF, name="onesrow_sb")
                nc.sync.dma_start(cosT[:], cost[:])
                nc.sync.dma_start(sinT[:], sint[:])
                nc.sync.dma_start(pswap_sb[:], pswap[:])
                nc.sync.dma_start(onescol_sb[:], onescol[:])
                nc.sync.dma_start(onesrow_sb[:], onesrow[:])
                epsq = tblp.tile([1, 1], F32, name="epsq")
                epsk = tblp.tile([1, 1], F32, name="epsk")
                nc.vector.memset(epsq[:], float(HD * EPS))
                nc.vector.memset(epsk[:], float(EPS))

                for ch in range(NCH):
                    b, half = ch // 2, ch % 2
                    lo, hi = half * CH, (half + 1) * CH
                    xts = []
                    for d in range(NDC):
                        t = xtp.tile([128, CH], BF, name=f"xt_{ch}_{d}", tag="xt")
                        nc.sync.dma_start(t[:], xt[d * 128:(d + 1) * 128,
                                                   ch * CH:(ch + 1) * CH])
                        xts.append(t)
                    for row in range(6):
                        ps = praw_p.tile([128, CH], F32, name=f"praw_{ch}_{row}", tag="praw")
                        for d in range(NDC):
                            if row < HPC:
                                lhsT = wq_sb[:, d * 512 + row * 128: d * 512 + (row + 1) * 128]
                            elif row == HPC:
                                lhsT = wk_sb[:, d * 128:(d + 1) * 128]
                            else:
                                lhsT = wv_sb[:, d * 128:(d + 1) * 128]
                            nc.tensor.matmul(ps[:], lhsT, xts[d][:],
                                             start=(d == 0), stop=(d == NDC - 1))
                        if row == 5:
                            nc.scalar.copy(vT[b][:, lo:hi], ps[:])
                            continue
                        target = qn[(b, row)] if row < HPC else kn[b]
                        raw_bf = ep.tile([128, CH], BF, name=f"rawbf_{ch}_{row}", tag="rawbf")
                        nc.scalar.copy(raw_bf[:], ps[:])
                        ps_sw = paux_p.tile([128, CH], F32, name=f"psw_{ch}_{row}", tag="psw")
                        nc.tensor.matmul(ps_sw[:], pswap_sb[:], raw_bf[:],
                                         start=True, stop=True)
                        t1 = ep.tile([128, CH], F32, name=f"t1_{ch}_{row}", tag="t1")
                        nc.vector.tensor_tensor(t1[:], ps[:], cosT[:, lo:hi], op=MULT)
                        t2 = ep.tile([128, CH], F32, name=f"t2_{ch}_{row}", tag="t2")
                        nc.vector.tensor_tensor(t2[:], ps_sw[:], sinT[:, lo:hi], op=MULT)
                        rope = ep.tile([128, CH], F32, name=f"rope_{ch}_{row}", tag="rope")
                        nc.vector.tensor_tensor(rope[:], t1[:], t2[:], op=ADD)
                        sq = ep.tile([128, CH], BF, name=f"sq_{ch}_{row}", tag="sq")
                        nc.vector.tensor_tensor(sq[:], rope[:], rope[:], op=MULT)
                        ps_ss = psml_p.tile([1, CH], F32, name=f"pss_{ch}_{row}", tag="pss")
                        nc.tensor.matmul(ps_ss[:], onescol_sb[:], sq[:],
                                         start=True, stop=True)
                        std = ep.tile([1, CH], F32, name=f"std_{ch}_{row}", tag="std")
                        # q absorbs the 1/sqrt(HD) attention scale:
                        #   q_n = rope / (sqrt(mean_sq + eps) * 128**0.25 * ... )
                        # score = (q.k)/sqrt(HD); fold into q: divide by
                        # sqrt(HD)**0.5 each? Instead: q_n = rope / sqrt(ss + HD*eps)
                        # gives rmsnorm(rope)/sqrt(HD).
                        if row < HPC:
                            nc.scalar.activation(std[:], ps_ss[:], Sqrt,
                                                 bias=epsq[:], scale=1.0)
                        else:
                            nc.scalar.activation(std[:], ps_ss[:], Sqrt,
                                                 bias=epsk[:], scale=1.0 / HD)
                        rstd = ep.tile([1, CH], F32, name=f"rstd_{ch}_{row}", tag="rstd")
                        nc.vector.reciprocal(rstd[:], std[:])
                        rstd_bf = ep.tile([1, CH], BF, name=f"rstdb_{ch}_{row}", tag="rstdb")
                        nc.scalar.copy(rstd_bf[:], rstd[:])
                        ps_bc = paux_p.tile([128, CH], F32, name=f"pbc_{ch}_{row}", tag="pbc")
                        nc.tensor.matmul(ps_bc[:], onesrow_sb[:], rstd_bf[:],
                                         start=True, stop=True)
                        nc.vector.tensor_tensor(target[:, lo:hi], rope[:], ps_bc[:], op=MULT)

            # ---------------- Phase 2: attention ----------------------------
            with (
                tc.tile_pool(name="wop", bufs=1) as wop,
                tc.tile_pool(name="c2", bufs=1) as c2p,
                tc.tile_pool(name="sout", bufs=3) as soutp,
            ):
                wo_sb = wop.tile([128, HPC * D], BF, name="wo_sb")
                for h in range(HPC):
                    nc.sync.dma_start(wo_sb[:, h * D:(h + 1) * D],
                                      wo[h * 128:(h + 1) * 128, :])
                maskd_sb = c2p.tile([128, 128], F32, name="maskd_sb")
                ident_sb = c2p.tile([128, 128], BF, name="ident_sb")
                nc.sync.dma_start(maskd_sb[:], maskd[:])
                nc.sync.dma_start(ident_sb[:], ident[:])

                with (
                    tc.tile_pool(name="sa", bufs=3) as sa,
                    tc.tile_pool(name="saT", bufs=16) as saT,
                    tc.tile_pool(name="psc", bufs=2, space="PSUM") as psc_p,
                    tc.tile_pool(name="ptr", bufs=2, space="PSUM") as ptr_p,
                    tc.tile_pool(name="pctx", bufs=2, space="PSUM") as pctx_p,
                ):
                    for b in range(B):
                        for j in range(NB):
                            ptv = ptr_p.tile([128, 128], BF, name=f"ptv_{b}_{j}", tag="ptr")
                            nc.tensor.transpose(ptv[:], vT[b][:, j * 128:(j + 1) * 128],
                                                ident_sb[:])
                            nc.scalar.copy(vnat[b][:, j * 128:(j + 1) * 128], ptv[:])

                    for b in range(B):
                        for h in range(HPC):
                            for i in range(NB):
                                W = (i + 1) * 128
                                psc = psc_p.tile([128, S], F32, name=f"psc_{b}_{h}_{i}", tag="psc")
                                for sc in range((W + 511) // 512):
                                    N = min(512, W - sc * 512)
                                    nc.tensor.matmul(
                                        psc[:, sc * 512: sc * 512 + N],
                                        qn[(b, h)][:, i * 128:(i + 1) * 128],
                                        kn[b][:, sc * 512: sc * 512 + N],
                                        start=True, stop=True)
                                nc.vector.tensor_tensor(psc[:, i * 128: W],
                                                        psc[:, i * 128: W],
                                                        maskd_sb[:], op=ADD)
                                negmax = sa.tile([128, 1], F32, name=f"ngm_{b}_{h}_{i}", tag="negmax")
                                nc.vector.tensor_reduce(negmax[:], psc[:, :W],
                                                        axis=AXX, op=MAX, negate=True)
                                attn_f = sa.tile([128, S], F32, name=f"af_{b}_{h}_{i}", tag="attnf")
                                denom = sa.tile([128, 1], F32, name=f"dn_{b}_{h}_{i}", tag="denom")
                                nc.scalar.activation(attn_f[:, :W], psc[:, :W], Exp,
                                                     bias=negmax[:], scale=1.0,
                                                     accum_out=denom[:])
                                recip = sa.tile([128, 1], F32, name=f"rc_{b}_{h}_{i}", tag="recip")
                                nc.vector.reciprocal(recip[:], denom[:])
                                attn_o = sa.tile([128, S], F32, name=f"ao_{b}_{h}_{i}", tag="attno")
                                nc.vector.tensor_scalar_mul(attn_o[:, :W], attn_f[:, :W], recip[:])
                                nc.sync.dma_start(attn_e[b, h, i * 128:(i + 1) * 128, 0:W],
                                                  attn_o[:, :W])
                                attn_bf = sa.tile([128, S], BF, name=f"ab_{b}_{h}_{i}", tag="attnbf")
                                nc.scalar.activation(attn_bf[:, :W], attn_f[:, :W], Copy,
                                                     scale=recip[:])
                                pctx = pctx_p.tile([128, 128], F32, name=f"pctx_{b}_{h}_{i}", tag="pctx")
                                for j in range(i + 1):
                                    pt = ptr_p.tile([128, 128], BF, name=f"pt_{b}_{h}_{i}_{j}", tag="ptr")
                                    nc.tensor.transpose(pt[:], attn_bf[:, j * 128:(j + 1) * 128],
                                                        ident_sb[:])
                                    aTj = saT.tile([128, 128], BF, name=f"aT_{b}_{h}_{i}_{j}", tag="aT")
                                    nc.scalar.copy(aTj[:], pt[:])
                                    nc.tensor.matmul(pctx[:],
                                                     vnat[b][:, j * 128:(j + 1) * 128],
                                                     aTj[:],
                                                     start=(j == 0), stop=(j == i))
                                nc.scalar.copy(ctxT[(b, h)][:, i * 128:(i + 1) * 128], pctx[:])

                # ------------- Phase 3: output projection (partial) ----------
                with tc.tile_pool(name="po", bufs=4, space="PSUM") as po_p:
                    for b in range(B):
                        for li in range(NB):
                            for dc in range(D // 512):
                                po = po_p.tile([128, 512], F32,
                                               name=f"po_{b}_{li}_{dc}", tag="po")
                                for h in range(HPC):
                                    nc.tensor.matmul(
                                        po[:],
                                        ctxT[(b, h)][:, li * 128:(li + 1) * 128],
                                        wo_sb[:, h * D + dc * 512: h * D + (dc + 1) * 512],
                                        start=(h == 0), stop=(h == HPC - 1))
                                ob = soutp.tile([128, 512], F32,
                                                name=f"ob_{b}_{li}_{dc}", tag="ob")
                                nc.vector.tensor_copy(ob[:], po[:])
                                nc.sync.dma_start(
                                    outp_e[b * S + li * 128: b * S + (li + 1) * 128,
                                           dc * 512:(dc + 1) * 512],
                                    ob[:])

    nc.compile()
    return nc


def prep_in_maps(x, wq, wk, wv, wo, freqs_cos, freqs_sin, mask, start_pos):
    x = np.asarray(x, dtype=np.float32)
    wq = np.asarray(wq, dtype=np.float32)
    wk = np.asarray(wk, dtype=np.float32)
    wv = np.asarray(wv, dtype=np.float32)
    wo = np.asarray(wo, dtype=np.float32)
    freqs_cos = np.asarray(freqs_cos, dtype=np.float32)
    freqs_sin = np.asarray(freqs_sin, dtype=np.float32)
    mask = np.asarray(mask, dtype=np.float32)
    sp = int(start_pos)

    xt = np.ascontiguousarray(x.reshape(B * S, D).T).astype(BF16)

    cos = freqs_cos[sp:sp + S]                       # (S, HD//2)
    sin = freqs_sin[sp:sp + S]
    cosT = np.repeat(cos.T, 2, axis=0).astype(np.float32)   # (HD, S)
    sinT = np.repeat(sin.T, 2, axis=0).astype(np.float32)
    sinT[0::2] *= -1.0
    cosT = np.ascontiguousarray(cosT)
    sinT = np.ascontiguousarray(sinT)

    maskd = np.ascontiguousarray(mask[0, 0, :128, :128]).astype(np.float32)

    pswap = np.zeros((128, 128), dtype=np.float32)
    idx = np.arange(128)
    pswap[idx, idx ^ 1] = 1.0
    pswap = pswap.astype(BF16)
    ident = np.eye(128, dtype=np.float32).astype(BF16)
    onescol = np.ones((128, 1), dtype=np.float32).astype(BF16)
    onesrow = np.ones((1, 128), dtype=np.float32).astype(BF16)
    chain = np.zeros((128, 4), dtype=np.float32)

    in_maps = []
    for c in range(NC):
        in_maps.append({
            "xt": xt,
            "wq": np.ascontiguousarray(wq[:, c * HPC * HD:(c + 1) * HPC * HD]).astype(BF16),
            "wk": np.ascontiguousarray(wk[:, c * HD:(c + 1) * HD]).astype(BF16),
            "wv": np.ascontiguousarray(wv[:, c * HD:(c + 1) * HD]).astype(BF16),
            "wo": np.ascontiguousarray(wo[c * HPC * HD:(c + 1) * HPC * HD, :]).astype(BF16),
            "cost": cosT,
            "sint": sinT,
            "maskd": maskd,
            "pswap": pswap,
            "ident": ident,
            "onescol": onescol,
            "onesrow": onesrow,
            "chain": chain,
        })
    return in_maps


def assemble(results):
    """results: list of 8 dicts with 'attn' (B,HPC,S,S) and 'outp' (B*S, D)."""
    attn = np.concatenate([np.asarray(r["attn"]) for r in results], axis=1)
    out = np.zeros((B * S, D), dtype=np.float32)
    for r in results:
        out += np.asarray(r["outp"])
    return out.reshape(B, S, D), attn


_NC_CACHE = None


def kernel(**inputs):
    global _NC_CACHE
    if _NC_CACHE is None:
        _NC_CACHE = build_nc()
    in_maps = prep_in_maps(**inputs)
    res = run_bass_kernel_spmd(_NC_CACHE, in_maps, list(range(NC)))
    return assemble(res.results)


# revision 9
# speedup vs baseline: 1183.0341x; 1183.0341x over previous
"""Trainium2 Bass kernel for GQA attention block (B=2,S=1024,D=4096,H=32,KVH=8,HD=128).

Tensor-parallel over heads across 8 NeuronCores: core c owns q-heads
[4c,4c+4), kv-head c, wo rows [512c, 512(c+1)). Attention is fully local
per core; host sums the 8 partial output projections and concatenates the
attention-probability shards over the head axis.
"""

import sys

sys.path.insert(0, "/opt/trn_rl_repo")

import math

import numpy as np
import ml_dtypes

import concourse.bass as bass
import concourse.bacc as bacc
import concourse.tile as tile
from concourse import mybir
from concourse.bass_utils import run_bass_kernel_spmd

BF16 = np.dtype(ml_dtypes.bfloat16)
F32 = mybir.dt.float32
BF = mybir.dt.bfloat16

B, S, D = 2, 1024, 4096
H, KVH, HD = 32, 8, 128
NC = 8
HPC = H // NC          # q heads per core = 4
EPS = 1e-5
SCALE = 1.0 / math.sqrt(HD)

Exp = mybir.ActivationFunctionType.Exp
Sqrt = mybir.ActivationFunctionType.Sqrt
Copy = mybir.ActivationFunctionType.Copy
ADD = mybir.AluOpType.add
MULT = mybir.AluOpType.mult
MAX = mybir.AluOpType.max
AXX = mybir.AxisListType.X


def build_nc(stub=False):
    nc = bacc.Bacc("TRN2", target_bir_lowering=False, debug=False,
                   enable_asserts=True, num_devices=NC)

    xt = nc.dram_tensor("xt", [D, B * S], BF, kind="ExternalInput").ap()
    wq = nc.dram_tensor("wq", [D, HPC * HD], BF, kind="ExternalInput").ap()
    wk = nc.dram_tensor("wk", [D, HD], BF, kind="ExternalInput").ap()
    wv = nc.dram_tensor("wv", [D, HD], BF, kind="ExternalInput").ap()
    wo = nc.dram_tensor("wo", [HPC * HD, D], BF, kind="ExternalInput").ap()
    cost = nc.dram_tensor("cost", [HD, S], F32, kind="ExternalInput").ap()
    sint = nc.dram_tensor("sint", [HD, S], F32, kind="ExternalInput").ap()
    maskd = nc.dram_tensor("maskd", [128, 128], F32, kind="ExternalInput").ap()
    pswap = nc.dram_tensor("pswap", [128, 128], BF, kind="ExternalInput").ap()
    ident = nc.dram_tensor("ident", [128, 128], BF, kind="ExternalInput").ap()
    onescol = nc.dram_tensor("onescol", [128, 1], BF, kind="ExternalInput").ap()
    onesrow = nc.dram_tensor("onesrow", [1, 128], BF, kind="ExternalInput").ap()
    chain = nc.dram_tensor("chain", [128, 4], F32, kind="ExternalInput").ap()

    attn_e = nc.dram_tensor("attn", [B, HPC, S, S], F32, kind="ExternalOutput").ap()
    outp_e = nc.dram_tensor("outp", [B * S, D], F32, kind="ExternalOutput").ap()
    chain_o = nc.dram_tensor("chain_out", [128, 4], F32, kind="ExternalOutput").ap()

    NB = S // 128          # 8 l/s blocks per batch
    NDC = D // 128         # 32 contraction chunks
    CH = 512               # seq chunk for projections
    NCH = B * S // CH      # 4 chunks

    if stub:
        with tile.TileContext(nc) as tc:
            with tc.tile_pool(name="chainp", bufs=1) as chp:
                cht = chp.tile([128, 4], F32, name="cht")
                nc.sync.dma_start(cht[:], chain[:])
                nc.sync.dma_start(chain_o[:], cht[:])
        nc.compile()
        return nc

    with tile.TileContext(nc) as tc:
        with tc.tile_pool(name="chainp", bufs=1) as chp:
            cht = chp.tile([128, 4], F32, name="cht")
            nc.sync.dma_start(cht[:], chain[:])
            nc.sync.dma_start(chain_o[:], cht[:])

        with tc.tile_pool(name="persist", bufs=1) as pp:
            qn = {(b, h): pp.tile([128, S], BF, name=f"qn_{b}_{h}", tag=f"qn_{b}_{h}")
                  for b in range(B) for h in range(HPC)}
            kn = {b: pp.tile([128, S], BF, name=f"kn_{b}", tag=f"kn_{b}") for b in range(B)}
            vT = {b: pp.tile([128, S], BF, name=f"vT_{b}", tag=f"vT_{b}") for b in range(B)}
            vnat = {b: pp.tile([128, S], BF, name=f"vnat_{b}", tag=f"vnat_{b}") for b in range(B)}
            ctxT = {(b, h): pp.tile([128, S], BF, name=f"ctxT_{b}_{h}", tag=f"ctxT_{b}_{h}")
                    for b in range(B) for h in range(HPC)}

            # ---------------- Phase 1: projections + RoPE + QK rmsnorm -------
            with (
                tc.tile_pool(name="xtp", bufs=64) as xtp,
                tc.tile_pool(name="wp", bufs=1) as wp,
                tc.tile_pool(name="tbl", bufs=1) as tblp,
                tc.tile_pool(name="ep", bufs=3) as ep,
                tc.tile_pool(name="praw", bufs=2, space="PSUM") as praw_p,
                tc.tile_pool(name="paux", bufs=2, space="PSUM") as paux_p,
                tc.tile_pool(name="psml", bufs=2, space="PSUM") as psml_p,
            ):
                wq_sb = wp.tile([128, NDC * HPC * HD], BF, name="wq_sb")
                wk_sb = wp.tile([128, NDC * HD], BF, name="wk_sb")
                wv_sb = wp.tile([128, NDC * HD], BF, name="wv_sb")
                for d in range(NDC):
                    nc.sync.dma_start(wq_sb[:, d * 512:(d + 1) * 512],
                                      wq[d * 128:(d + 1) * 128, :])
                    nc.sync.dma_start(wk_sb[:, d * 128:(d + 1) * 128],
                                      wk[d * 128:(d + 1) * 128, :])
                    nc.sync.dma_start(wv_sb[:, d * 128:(d + 1) * 128],
                                      wv[d * 128:(d + 1) * 128, :])
                cosT = tblp.tile([128, S], F32, name="cosT")
                sinT = tblp.tile([128, S], F32, name="sinT")
                pswap_sb = tblp.tile([128, 128], BF, name="pswap_sb")
                onescol_sb = tblp.tile([128, 1], BF, name="onescol_sb")
                onesrow_sb = tblp.tile([1, 128], BF, name="onesrow_sb")
                nc.sync.dma_start(cosT[:], cost[:])
                nc.sync.dma_start(sinT[:], sint[:])
                nc.sync.dma_start(pswap_sb[:], pswap[:])
                nc.sync.dma_start(onescol_sb[:], onescol[:])
                nc.sync.dma_start(onesrow_sb[:], onesrow[:])
                epsq = tblp.tile([1, 1], F32, name="epsq")
                epsk = tblp.tile([1, 1], F32, name="epsk")
                nc.vector.memset(epsq[:], float(HD * EPS))
                nc.vector.memset(epsk[:], float(EPS))

                for ch in range(NCH):
                    b, half = ch // 2, ch % 2
                    lo, hi = half * CH, (half + 1) * CH
                    xts = []
                    for d in range(NDC):
                        t = xtp.tile([128, CH], BF, name=f"xt_{ch}_{d}", tag="xt")
                        nc.sync.dma_start(t[:], xt[d * 128:(d + 1) * 128,
                                                   ch * CH:(ch + 1) * CH])
                        xts.append(t)
                    for row in range(6):
                        ps = praw_p.tile([128, CH], F32, name=f"praw_{ch}_{row}", tag="praw")
                        for d in range(NDC):
                            if row < HPC:
                                lhsT = wq_sb[:, d * 512 + row * 128: d * 512 + (row + 1) * 128]
                            elif row == HPC:
                                lhsT = wk_sb[:, d * 128:(d + 1) * 128]
                            else:
                                lhsT = wv_sb[:, d * 128:(d + 1) * 128]
                            nc.tensor.matmul(ps[:], lhsT, xts[d][:],
                                             start=(d == 0), stop=(d == NDC - 1))
                        if row == 5:
                            nc.scalar.copy(vT[b][:, lo:hi], ps[:])
                            continue
                        target = qn[(b, row)] if row < HPC else kn[b]
                        raw_bf = ep.tile([128, CH], BF, name=f"rawbf_{ch}_{row}", tag="rawbf")
                        nc.scalar.copy(raw_bf[:], ps[:])
                        ps_sw = paux_p.tile([128, CH], F32, name=f"psw_{ch}_{row}", tag="psw")
                        nc.tensor.matmul(ps_sw[:], pswap_sb[:], raw_bf[:],
                                         start=True, stop=True)
                        t1 = ep.tile([128, CH], F32, name=f"t1_{ch}_{row}", tag="t1")
                        nc.vector.tensor_tensor(t1[:], ps[:], cosT[:, lo:hi], op=MULT)
                        t2 = ep.tile([128, CH], F32, name=f"t2_{ch}_{row}", tag="t2")
                        nc.vector.tensor_tensor(t2[:], ps_sw[:], sinT[:, lo:hi], op=MULT)
                        rope = ep.tile([128, CH], F32, name=f"rope_{ch}_{row}", tag="rope")
                        nc.vector.tensor_tensor(rope[:], t1[:], t2[:], op=ADD)
                        sq = ep.tile([128, CH], BF, name=f"sq_{ch}_{row}", tag="sq")
                        nc.vector.tensor_tensor(sq[:], rope[:], rope[:], op=MULT)
                        ps_ss = psml_p.tile([1, CH], F32, name=f"pss_{ch}_{row}", tag="pss")
                        nc.tensor.matmul(ps_ss[:], onescol_sb[:], sq[:],
                                         start=True, stop=True)
                        std = ep.tile([1, CH], F32, name=f"std_{ch}_{row}", tag="std")
                        # q absorbs the 1/sqrt(HD) attention scale:
                        #   q_n = rope / (sqrt(mean_sq + eps) * 128**0.25 * ... )
                        # score = (q.k)/sqrt(HD); fold into q: divide by
                        # sqrt(HD)**0.5 each? Instead: q_n = rope / sqrt(ss + HD*eps)
                        # gives rmsnorm(rope)/sqrt(HD).
                        if row < HPC:
                            nc.scalar.activation(std[:], ps_ss[:], Sqrt,
                                                 bias=epsq[:], scale=1.0)
                        else:
                            nc.scalar.activation(std[:], ps_ss[:], Sqrt,
                                                 bias=epsk[:], scale=1.0 / HD)
                        rstd = ep.tile([1, CH], F32, name=f"rstd_{ch}_{row}", tag="rstd")
                        nc.vector.reciprocal(rstd[:], std[:])
                        rstd_bf = ep.tile([1, CH], BF, name=f"rstdb_{ch}_{row}", tag="rstdb")
                        nc.scalar.copy(rstd_bf[:], rstd[:])
                        ps_bc = paux_p.tile([128, CH], F32, name=f"pbc_{ch}_{row}", tag="pbc")
                        nc.tensor.matmul(ps_bc[:], onesrow_sb[:], rstd_bf[:],
                                         start=True, stop=True)
                        nc.vector.tensor_tensor(target[:, lo:hi], rope[:], ps_bc[:], op=MULT)

            # ---------------- Phase 2: attention ----------------------------
            with (
                tc.tile_pool(name="wop", bufs=1) as wop,
                tc.tile_pool(name="c2", bufs=1) as c2p,
                tc.tile_pool(name="sout", bufs=3) as soutp,
            ):
                wo_sb = wop.tile([128, HPC * D], BF, name="wo_sb")
                for h in range(HPC):
                    nc.sync.dma_start(wo_sb[:, h * D:(h + 1) * D],
                                      wo[h * 128:(h + 1) * 128, :])
                maskd_sb = c2p.tile([128, 128], F32, name="maskd_sb")
                ident_sb = c2p.tile([128, 128], BF, name="ident_sb")
                nc.sync.dma_start(maskd_sb[:], maskd[:])
                nc.sync.dma_start(ident_sb[:], ident[:])

                with (
                    tc.tile_pool(name="sa", bufs=3) as sa,
                    tc.tile_pool(name="saT", bufs=16) as saT,
                    tc.tile_pool(name="psc", bufs=2, space="PSUM") as psc_p,
                    tc.tile_pool(name="ptr", bufs=2, space="PSUM") as ptr_p,
                    tc.tile_pool(name="pctx", bufs=2, space="PSUM") as pctx_p,
                ):
                    for b in range(B):
                        for j in range(NB):
                            ptv = ptr_p.tile([128, 128], BF, name=f"ptv_{b}_{j}", tag="ptr")
                            nc.tensor.transpose(ptv[:], vT[b][:, j * 128:(j + 1) * 128],
                                                ident_sb[:])
                            nc.scalar.copy(vnat[b][:, j * 128:(j + 1) * 128], ptv[:])

                    for b in range(B):
                        for h in range(HPC):
                            for i in range(NB):
                                W = (i + 1) * 128
                                psc = psc_p.tile([128, S], F32, name=f"psc_{b}_{h}_{i}", tag="psc")
                                for sc in range((W + 511) // 512):
                                    N = min(512, W - sc * 512)
                                    nc.tensor.matmul(
                                        psc[:, sc * 512: sc * 512 + N],
                                        qn[(b, h)][:, i * 128:(i + 1) * 128],
                                        kn[b][:, sc * 512: sc * 512 + N],
                                        start=True, stop=True)
                                nc.vector.tensor_tensor(psc[:, i * 128: W],
                                                        psc[:, i * 128: W],
                                                        maskd_sb[:], op=ADD)
                                negmax = sa.tile([128, 1], F32, name=f"ngm_{b}_{h}_{i}", tag="negmax")
                                nc.vector.tensor_reduce(negmax[:], psc[:, :W],
                                                        axis=AXX, op=MAX, negate=True)
                                attn_f = sa.tile([128, S], F32, name=f"af_{b}_{h}_{i}", tag="attnf")
                                denom = sa.tile([128, 1], F32, name=f"dn_{b}_{h}_{i}", tag="denom")
                                nc.scalar.activation(attn_f[:, :W], psc[:, :W], Exp,
                                                     bias=negmax[:], scale=1.0,
                                                     accum_out=denom[:])
                                recip = sa.tile([128, 1], F32, name=f"rc_{b}_{h}_{i}", tag="recip")
                                nc.vector.reciprocal(recip[:], denom[:])
                                attn_o = sa.tile([128, S], F32, name=f"ao_{b}_{h}_{i}", tag="attno")
                                nc.vector.tensor_scalar_mul(attn_o[:, :W], attn_f[:, :W], recip[:])
                                nc.sync.dma_start(attn_e[b, h, i * 128:(i + 1) * 128, 0:W],
                                                  attn_o[:, :W])
                                attn_bf = sa.tile([128, S], BF, name=f"ab_{b}_{h}_{i}", tag="attnbf")
                                nc.scalar.activation(attn_bf[:, :W], attn_f[:, :W], Copy,
                                                     scale=recip[:])
                                pctx = pctx_p.tile([128, 128], F32, name=f"pctx_{b}_{h}_{i}", tag="pctx")
                                for j in range(i + 1):
                                    pt = ptr_p.tile([128, 128], BF, name=f"pt_{b}_{h}_{i}_{j}", tag="ptr")
                                    nc.tensor.transpose(pt[:], attn_bf[:, j * 128:(j + 1) * 128],
                                                        ident_sb[:])
                                    aTj = saT.tile([128, 128], BF, name=f"aT_{b}_{h}_{i}_{j}", tag="aT")
                                    nc.scalar.copy(aTj[:], pt[:])
                                    nc.tensor.matmul(pctx[:],
                                                     vnat[b][:, j * 128:(j + 1) * 128],
                                                     aTj[:],
                                                     start=(j == 0), stop=(j == i))
                                nc.scalar.copy(ctxT[(b, h)][:, i * 128:(i + 1) * 128], pctx[:])

                # ------------- Phase 3: output projection (partial) ----------
                with tc.tile_pool(name="po", bufs=4, space="PSUM") as po_p:
                    for b in range(B):
                        for li in range(NB):
                            for dc in range(D // 512):
                                po = po_p.tile([128, 512], F32,
                                               name=f"po_{b}_{li}_{dc}", tag="po")
                                for h in range(HPC):
                                    nc.tensor.matmul(
                                        po[:],
                                        ctxT[(b, h)][:, li * 128:(li + 1) * 128],
                                        wo_sb[:, h * D + dc * 512: h * D + (dc + 1) * 512],
                                        start=(h == 0), stop=(h == HPC - 1))
                                ob = soutp.tile([128, 512], F32,
                                                name=f"ob_{b}_{li}_{dc}", tag="ob")
                                nc.vector.tensor_copy(ob[:], po[:])
                                nc.sync.dma_start(
                                    outp_e[b * S + li * 128: b * S + (li + 1) * 128,
                                           dc * 512:(dc + 1) * 512],
                                    ob[:])

    nc.compile()
    return nc


def prep_in_maps(x, wq, wk, wv, wo, freqs_cos, freqs_sin, mask, start_pos):
    x = np.asarray(x, dtype=np.float32)
    wq = np.asarray(wq, dtype=np.float32)
    wk = np.asarray(wk, dtype=np.float32)
    wv = np.asarray(wv, dtype=np.float32)
    wo = np.asarray(wo, dtype=np.float32)
    freqs_cos = np.asarray(freqs_cos, dtype=np.float32)
    freqs_sin = np.asarray(freqs_sin, dtype=np.float32)
    mask = np.asarray(mask, dtype=np.float32)
    sp = int(start_pos)

    xt = np.ascontiguousarray(x.reshape(B * S, D).T).astype(BF16)

    cos = freqs_cos[sp:sp + S]                       # (S, HD//2)
    sin = freqs_sin[sp:sp + S]
    cosT = np.repeat(cos.T, 2, axis=0).astype(np.float32)   # (HD, S)
    sinT = np.repeat(sin.T, 2, axis=0).astype(np.float32)
    sinT[0::2] *= -1.0
    cosT = np.ascontiguousarray(cosT)
    sinT = np.ascontiguousarray(sinT)

    maskd = np.ascontiguousarray(mask[0, 0, :128, :128]).astype(np.float32)

    pswap = np.zeros((128, 128), dtype=np.float32)
    idx = np.arange(128)
    pswap[idx, idx ^ 1] = 1.0
    pswap = pswap.astype(BF16)
    ident = np.eye(128, dtype=np.float32).astype(BF16)
    onescol = np.ones((128, 1), dtype=np.float32).astype(BF16)
    onesrow = np.ones((1, 128), dtype=np.float32).astype(BF16)
    chain = np.zeros((128, 4), dtype=np.float32)

    in_maps = []
    for c in range(NC):
        in_maps.append({
            "xt": xt,
            "wq": np.ascontiguousarray(wq[:, c * HPC * HD:(c + 1) * HPC * HD]).astype(BF16),
            "wk": np.ascontiguousarray(wk[:, c * HD:(c + 1) * HD]).astype(BF16),
            "wv": np.ascontiguousarray(wv[:, c * HD:(c + 1) * HD]).astype(BF16),
            "wo": np.ascontiguousarray(wo[c * HPC * HD:(c + 1) * HPC * HD, :]).astype(BF16),
            "cost": cosT,
            "sint": sinT,
            "maskd": maskd,
            "pswap": pswap,
            "ident": ident,
            "onescol": onescol,
            "onesrow": onesrow,
            "chain": chain,
        })
    return in_maps


def assemble(results):
    """results: list of 8 dicts with 'attn' (B,HPC,S,S) and 'outp' (B*S, D)."""
    attn = np.concatenate([np.asarray(r["attn"]) for r in results], axis=1)
    out = np.zeros((B * S, D), dtype=np.float32)
    for r in results:
        out += np.asarray(r["outp"])
    return out.reshape(B, S, D), attn


_NC_CACHE = None


def kernel(**inputs):
    global _NC_CACHE
    if _NC_CACHE is None:
        _NC_CACHE = build_nc()
    in_maps = prep_in_maps(**inputs)
    res = run_bass_kernel_spmd(_NC_CACHE, in_maps, list(range(NC)))
    return assemble(res.results)
